# revision 1
# baseline (speedup 1.0000x reference)
"""Trainium2 Bass kernel: GQA attention (B=2, S=2048, D=2048, 32 q-heads,
8 kv-heads, head_dim 64, RoPE interleaved, causal) on 8 NeuronCores.

Sharding: tensor-parallel over heads. Core c owns q-heads 4c..4c+3 (= kv head
c) for BOTH batch elements, computes q/k/v projections + RoPE + causal
attention for those heads over all queries, then an 8-core mesh AllToAll
redistributes attention outputs so core c holds all 2048 head-dims for its
(batch, query-quarter) = (c//4, c%4) row block and computes the output
projection locally. The kernel returns out^T row-shards; the host only
transposes/concatenates.

All matmuls run as float32r (TF32-like, FP22 mantissa) on the PE at full
rate. exp runs on ScalarE with the 1/sqrt(hd) scale folded into the
activation's affine pre-scale. Softmax denominators come from ones-column
matmuls accumulated alongside attn@V; normalization is a DVE reciprocal plus
a K=1 broadcast matmul.
"""

import numpy as np

B, S, D = 2, 2048, 2048
NH, NKV, HD = 32, 8, 64
THETA = 10000.0
NCORES = 8
NEG = -1.0e30

_BUILT = None


def _swap_mask():
    m = []
    for i in range(16):
        m += [2 * i + 1, 2 * i]
    return m


def _build():
    """Build + compile the SPMD Bass program (once per process)."""
    global _BUILT
    if _BUILT is not None:
        return _BUILT

    from contextlib import ExitStack

    import concourse.tile as tile
    from concourse import bacc, mybir

    f32 = mybir.dt.float32
    f32r = mybir.dt.float32r
    AF = mybir.ActivationFunctionType

    nc = bacc.Bacc(
        "TRN2", target_bir_lowering=False, debug=False, num_devices=NCORES
    )

    xT = nc.dram_tensor("xT", [B, 16, 2, 128, 1024], f32r, kind="ExternalInput").ap()
    wqTc = nc.dram_tensor("wqTc", [D, 256], f32r, kind="ExternalInput").ap()
    wkvTc = nc.dram_tensor("wkvTc", [D, 128], f32r, kind="ExternalInput").ap()
    woT = nc.dram_tensor("woT", [D, D], f32r, kind="ExternalInput").ap()
    cosd = nc.dram_tensor("cosd", [128, B * S], f32, kind="ExternalInput").ap()
    sind = nc.dram_tensor("sind", [128, B * S], f32, kind="ExternalInput").ap()
    maskA = nc.dram_tensor("maskA", [128, 256], f32, kind="ExternalInput").ap()
    maskB = nc.dram_tensor("maskB", [128, 256], f32, kind="ExternalInput").ap()
    onesd = nc.dram_tensor("onesd", [128, 64], f32r, kind="ExternalInput").ap()
    outT = nc.dram_tensor("outT", [D, 512], f32, kind="ExternalOutput").ap()

    SW = _swap_mask()
    SCALE = 1.0 / float(np.sqrt(HD))

    with tile.TileContext(nc) as tc, ExitStack() as top:
        top.enter_context(
            nc.allow_low_precision(reason="fp32r (FP22) matmul inputs by design")
        )
        wor = top.enter_context(tc.tile_pool(name="wores", bufs=1))
        wo_t = [wor.tile([128, D], f32r, tag=f"wo{e}", name=f"wo{e}") for e in range(8)]

        ph12 = top.enter_context(ExitStack())
        res = ph12.enter_context(tc.tile_pool(name="resident", bufs=1))
        # RoPE'd projections, resident across phases
        qt = [res.tile([128, B * S], f32r, tag=f"qt{p}", name=f"qt{p}") for p in range(2)]
        kt = res.tile([128, B * S], f32r, tag="kt")  # kv head, duplicated rows
        vt = [res.tile([128, HD + 1], f32r, tag=f"vt{i}", name=f"vt{i}") for i in range(2 * 16)]
        mA = res.tile([128, 256], f32, tag="mA")
        mB = res.tile([128, 256], f32, tag="mB")
        ones_t = res.tile([128, 64], f32r, tag="ones")

        nc.sync.dma_start(out=mA[:], in_=maskA[:])
        nc.sync.dma_start(out=mB[:], in_=maskB[:])
        nc.sync.dma_start(out=ones_t[:], in_=onesd[:])

        dram = top.enter_context(tc.tile_pool(name="dram", bufs=1, space="DRAM"))
        a2a_in = [dram.tile([8, 256, 256], f32r, tag=f"a2ain{h}", name=f"a2ain{h}") for h in range(2)]
        a2a_out = [dram.tile([8, 256, 256], f32r, tag=f"a2aout{h}", name=f"a2aout{h}") for h in range(2)]

        # ---------------- phase 1: projections + RoPE -------------------
        with ExitStack() as ph1:
            wres = ph1.enter_context(tc.tile_pool(name="wres", bufs=1))
            cos_t = wres.tile([128, B * S], f32, tag="cos")
            sin_t = wres.tile([128, B * S], f32, tag="sin")
            ident = wres.tile([128, 128], f32, tag="ident")
            wq_t = [wres.tile([128, 256], f32r, tag=f"wq{d}", name=f"wq{d}") for d in range(16)]
            wkv_t = [wres.tile([128, 128], f32r, tag=f"wkv{d}", name=f"wkv{d}") for d in range(16)]
            for d in range(16):
                nc.sync.dma_start(out=wq_t[d][:], in_=wqTc[128 * d:128 * (d + 1), :])
                nc.sync.dma_start(out=wkv_t[d][:], in_=wkvTc[128 * d:128 * (d + 1), :])
            nc.sync.dma_start(out=cos_t[:], in_=cosd[:])
            nc.sync.dma_start(out=sin_t[:], in_=sind[:])
            from concourse.masks import make_identity

            make_identity(nc, ident[:])

            xp = ph1.enter_context(tc.tile_pool(name="xchunk", bufs=4))
            pp = ph1.enter_context(tc.tile_pool(name="projpsum", bufs=1, space="PSUM"))
            tvp = ph1.enter_context(tc.tile_pool(name="vtpsum", bufs=2, space="PSUM"))
            vstage = ph1.enter_context(tc.tile_pool(name="vstage", bufs=2))
            rtmp = ph1.enter_context(tc.tile_pool(name="ropetmp", bufs=1))

            for bh in range(4):
                b, half = bh // 2, bh % 2
                qcs = [1024 * half, 1024 * half + 512]
                cols = [2048 * b + qc for qc in qcs]
                pq = [
                    [pp.tile([128, 512], f32, tag=f"q{s}{i}", name=f"pq{bh}{s}{i}") for i in range(2)]
                    for s in range(2)
                ]
                pkv = [pp.tile([128, 512], f32, tag=f"kv{s}", name=f"pkv{bh}{s}") for s in range(2)]
                for d in range(16):
                    xt_ = xp.tile([128, 1024], f32r, tag="x")
                    nc.sync.dma_start(out=xt_[:], in_=xT[b, d, half])
                    st, sp_ = (d == 0), (d == 15)
                    for s in range(2):
                        xs = xt_[:, 512 * s:512 * s + 512]
                        nc.tensor.matmul(pq[s][0][:], wq_t[d][:, 0:128], xs, start=st, stop=sp_)
                        nc.tensor.matmul(pq[s][1][:], wq_t[d][:, 128:256], xs, start=st, stop=sp_)
                        nc.tensor.matmul(pkv[s][:], wkv_t[d][:], xs, start=st, stop=sp_)

                for s in range(2):
                    col = cols[s]
                    # v first: frees the PE transposes to run while DVE does RoPE
                    vs = vstage.tile([128, 512], f32, tag="vs")
                    nc.vector.tensor_copy(vs[64:128, :], pkv[s][64:128, :])
                    for j in range(4):
                        ptv = tvp.tile([128, HD], f32, tag="tv")
                        nc.tensor.transpose(
                            ptv[:], vs[64:128, 128 * j:128 * (j + 1)], ident[64:128, 64:128]
                        )
                        kb = 4 * (2 * half + s) + j
                        nc.vector.tensor_copy(vt[16 * b + kb][:, 0:HD], ptv[:])
                        nc.sync.dma_start(
                            out=vt[16 * b + kb][:, HD:HD + 1], in_=onesd[:, 0:1]
                        )

                    # RoPE on q head-pairs
                    for p in range(2):
                        ps = pq[s][p]
                        t1 = rtmp.tile([128, 512], f32, tag="t1")
                        nc.vector.tensor_mul(t1[:], ps[:], cos_t[:, col:col + 512])
                        sw = rtmp.tile([128, 512], f32, tag="sw")
                        nc.vector.stream_shuffle(sw[:], ps[:], SW)
                        t2 = rtmp.tile([128, 512], f32, tag="t2")
                        nc.vector.tensor_mul(t2[:], sw[:], sin_t[:, col:col + 512])
                        nc.vector.tensor_add(qt[p][:, col:col + 512], t1[:], t2[:])

                    # RoPE on k (kv psum rows 0:64), then duplicate to rows 64:128
                    t1 = rtmp.tile([128, 512], f32, tag="t1")
                    nc.vector.tensor_mul(t1[0:64, :], pkv[s][0:64, :], cos_t[0:64, col:col + 512])
                    sw = rtmp.tile([128, 512], f32, tag="sw")
                    nc.vector.stream_shuffle(sw[0:64, :], pkv[s][0:64, :], SW)
                    t2 = rtmp.tile([128, 512], f32, tag="t2")
                    nc.vector.tensor_mul(t2[0:64, :], sw[0:64, :], sin_t[0:64, col:col + 512])
                    nc.vector.tensor_add(kt[0:64, col:col + 512], t1[0:64, :], t2[0:64, :])
                    nc.sync.dma_start(
                        out=kt[64:128, col:col + 512], in_=kt[0:64, col:col + 512]
                    )


        for e in range(8):
            nc.sync.dma_start(out=wo_t[e][:], in_=woT[128 * e:128 * (e + 1), :])

        # ---------------- phase 2: causal attention ---------------------
        with ExitStack() as ph2:
            spp = ph2.enter_context(tc.tile_pool(name="scorepsum", bufs=3, space="PSUM"))
            avp = ph2.enter_context(tc.tile_pool(name="avpsum", bufs=2, space="PSUM"))
            esp = ph2.enter_context(tc.tile_pool(name="expsbuf", bufs=3))
            nrm = ph2.enter_context(tc.tile_pool(name="normtmp", bufs=2))
            nrm1 = ph2.enter_context(tc.tile_pool(name="normtmp1", bufs=1))

            qv = [
                qt[p][:].rearrange("p (b h u i) -> p b h u i", b=2, h=2, u=4)
                for p in range(2)
            ]

            groups = [
                (b, u, p)
                for u in [0, 2, 1, 3]
                for b in range(B)
                for p in range(2)
            ]

            def make_stage(b, u, p, av):
                nkb = 2 * u + 10

                def scores(kb):
                    kcol = 2048 * b + 128 * kb
                    full = kb <= 2 * u + 1
                    sp = spp.tile([128, 1024], f32, tag="sp", name=f"sp{b}{u}{p}{kb}")
                    for hh in range(2):
                        r0 = 64 * hh
                        lhs = kt[r0:r0 + 64, kcol:kcol + 128]
                        if full:
                            rhs = qv[p][r0:r0 + 64, b, :, u, :]
                            nc.tensor.matmul(
                                sp[:, 512 * hh:512 * hh + 512], lhs, rhs,
                                start=True, stop=True,
                            )
                        else:
                            rhs = qv[p][r0:r0 + 64, b, 1, u, :]
                            nc.tensor.matmul(
                                sp[:, 512 * hh + 256:512 * hh + 512], lhs, rhs,
                                start=True, stop=True,
                            )
                    msk = None
                    if kb == 2 * u:
                        msk, mlo = mA, True
                    elif kb == 2 * u + 1:
                        msk, mlo = mB, True
                    elif kb == 2 * u + 8:
                        msk, mlo = mA, False
                    elif kb == 2 * u + 9:
                        msk, mlo = mB, False
                    if msk is not None:
                        off = 0 if mlo else 256
                        for hh in range(2):
                            c0 = 512 * hh + off
                            nc.vector.tensor_add(
                                sp[:, c0:c0 + 256], sp[:, c0:c0 + 256], msk[:]
                            )
                    return sp

                def expav(kb, sp, first, last):
                    full = kb <= 2 * u + 1
                    ex = esp.tile([128, 1024], f32r, tag="ex", name=f"ex{b}{u}{p}{kb}")
                    if full:
                        nc.scalar.activation(ex[:], sp[:], AF.Exp, scale=SCALE)
                    else:
                        spv = sp[:].rearrange("p (h i) -> p h i", h=2)[:, :, 256:512]
                        exv = ex[:].rearrange("p (h i) -> p h i", h=2)[:, :, 256:512]
                        nc.scalar.activation(exv, spv, AF.Exp, scale=SCALE)
                    v_ = vt[16 * b + kb]
                    for hh in range(2):
                        e0 = 512 * hh
                        if full:
                            # start=True clears has_written BANK-wide: only the
                            # very first MM in the tile sets it
                            nc.tensor.matmul(
                                av[hh][:, 0:512], v_[:], ex[:, e0:e0 + 512],
                                start=first, stop=last,
                            )
                        else:
                            nc.tensor.matmul(
                                av[hh][:, 256:512], v_[:], ex[:, e0 + 256:e0 + 512],
                                start=first, stop=last,
                            )

                def avcopy():
                    cps = []
                    for hh in range(2):
                        cp = nrm.tile([65, 512], f32, tag="cp", name=f"cp{b}{u}{p}{hh}")
                        nc.vector.tensor_copy(cp[:], av[hh][0:65, :])
                        cps.append(cp)
                    return cps

                def recip(cps):
                    rrs = []
                    for hh in range(2):
                        rr = nrm.tile([128, 512], f32r, tag="rr", name=f"rr{b}{u}{p}{hh}")
                        nc.vector.reciprocal(rr[64:65, :], cps[hh][64:65, :])
                        rrs.append(rr)
                    return rrs

                def rest(cps, rrs):
                    for hh in range(2):
                        bc = spp.tile([128, 512], f32, tag="sp", name=f"bc{b}{u}{p}{hh}")
                        nc.tensor.matmul(
                            bc[0:64, :], ones_t[64:65, :], rrs[hh][64:65, :],
                            start=True, stop=True,
                        )
                        bcs = nrm1.tile([128, 512], f32, tag="bcs", name=f"bcs{b}{u}{p}{hh}")
                        nc.vector.tensor_copy(bcs[0:64, :], bc[0:64, :])
                        at_ = nrm.tile([64, 512], f32r, tag="at", name=f"at{b}{u}{p}{hh}")
                        nc.vector.tensor_mul(at_[:], cps[hh][0:64, :], bcs[0:64, :])
                        for hf in range(2):
                            dst = 4 * b + 2 * hf + u // 2
                            nc.sync.dma_start(
                                out=a2a_in[u % 2][
                                    dst, 128 * p + 64 * hh:128 * p + 64 * hh + 64, :
                                ],
                                in_=at_[:, 256 * hf:256 * hf + 256],
                            )

                return scores, expav, avcopy, recip, rest, nkb

            def emit_a2a(h):
                nc.gpsimd.collective_compute(
                    "AllToAll",
                    mybir.AluOpType.bypass,
                    replica_groups=[list(range(8))],
                    ins=[a2a_in[h][:].opt()],
                    outs=[a2a_out[h][:].opt()],
                )

            pending_recip = None
            pending_rest = None
            for gi, (b, u, p) in enumerate(groups):
                av = [
                    avp.tile([HD + 1, 512], f32, tag="av", name=f"av{b}{u}{p}{hh}")
                    for hh in range(2)
                ]
                scores, expav, avcopy, recip, rest, nkb = make_stage(b, u, p, av)
                diag = [2 * u, 2 * u + 1, 2 * u + 8, 2 * u + 9]
                order = [k for k in range(nkb) if k not in diag] + diag
                pipe = []
                for i, kb in enumerate(order):
                    sp = scores(kb)
                    pipe.append((kb, sp))
                    if i == 2 and pending_recip is not None:
                        rrs = pending_recip()
                        pending_recip = None
                        pending_rest = (lambda rrs=rrs: _rest_holder[0](rrs))
                    if i == 8 and pending_rest is not None:
                        pending_rest()
                        pending_rest = None
                        if gi == 8:
                            emit_a2a(0)  # first-half attn outputs all written
                    if len(pipe) > 2:
                        pk, psp = pipe.pop(0)
                        expav(pk, psp, first=(pk == order[0]), last=(pk == order[-1]))
                for pk, psp in pipe:
                    expav(pk, psp, first=(pk == order[0]), last=(pk == order[-1]))
                cps = avcopy()
                _rest_holder = [rest]
                pending_recip = (lambda recip=recip, rest=rest, cps=cps: recip(cps))
                _rest_holder[0] = (lambda rest=rest, cps=cps: (lambda rrs: rest(cps, rrs)))()
            rrs = pending_recip()
            _rest_holder[0](rrs)
            emit_a2a(1)


        ph12.close()

        # ---------------- phase 3: output projection --------------------
        with ExitStack() as ph3:
            wrh = ph3.enter_context(tc.tile_pool(name="worh", bufs=1))
            wo_t2 = [wrh.tile([128, D], f32r, tag=f"wo2{e}", name=f"wo2{e}") for e in range(8)]
            rh_t = [
                [wrh.tile([128, 256], f32r, tag=f"rh{h}{e}", name=f"rh{h}{e}") for e in range(16)]
                for h in range(2)
            ]
            for e in range(16):
                nc.sync.dma_start(
                    out=rh_t[0][e][:],
                    in_=a2a_out[0][e // 2, 128 * (e % 2):128 * (e % 2) + 128, :],
                )
            for e in range(8):
                nc.sync.dma_start(out=wo_t2[e][:], in_=woT[128 * (e + 8):128 * (e + 9), :])
            for e in range(16):
                nc.sync.dma_start(
                    out=rh_t[1][e][:],
                    in_=a2a_out[1][e // 2, 128 * (e % 2):128 * (e % 2) + 128, :],
                )
            wo_t = wo_t + wo_t2
            wop = ph3.enter_context(tc.tile_pool(name="wopsum", bufs=2, space="PSUM"))
            wos = ph3.enter_context(tc.tile_pool(name="wosbuf", bufs=2))
            warm0 = wop.tile([128, 256], f32, tag="warm", name="warmbank0")
            for w in range(56):
                nc.tensor.matmul(
                    warm0[:], wo_t[0][:, 0:128], rh_t[0][0][:],
                    start=True, stop=True, skip_group_check=True,
                )
            for h in range(2):
                if h == 1:
                    # keep the PE clock warm (HAM) across the AllToAll-2 wait;
                    # junk matmuls into a scratch bank, no consumers
                    warm = wop.tile([128, 256], f32, tag="warm", name="warmbank")
                    for w in range(200):
                        nc.tensor.matmul(
                            warm[:], wo_t[0][:, 0:128], rh_t[0][0][:],
                            start=True, stop=True, skip_group_check=True,
                        )
                for m in range(16):
                    po = wop.tile([128, 256], f32, tag="wo")
                    for e in range(16):
                        nc.tensor.matmul(
                            po[:], wo_t[e][:, 128 * m:128 * (m + 1)], rh_t[h][e][:],
                            start=(e == 0), stop=(e == 15),
                        )
                    os_ = wos.tile([128, 256], f32, tag="os")
                    nc.vector.tensor_copy(os_[:], po[:])
                    nc.sync.dma_start(
                        out=outT[128 * m:128 * (m + 1), 256 * h:256 * h + 256], in_=os_[:]
                    )

    nc.compile()
    _BUILT = nc
    return nc


def _host_inputs(x, wq, wk, wv, wo):
    """Per-core input maps (host-side layout prep only, no math on x)."""
    x = np.ascontiguousarray(x, dtype=np.float32)
    xT3 = x.transpose(0, 2, 1)
    xT = np.ascontiguousarray(
        xT3.reshape(B, 16, 128, 2, 1024).transpose(0, 1, 3, 2, 4)
    )
    woT = np.ascontiguousarray(np.asarray(wo, np.float32).T)

    inv = THETA ** (-np.arange(32, dtype=np.float64) / 32.0)
    ang = np.outer(inv, np.arange(S, dtype=np.float64))  # [32, S]
    cos1 = np.cos(ang).astype(np.float32)
    sin1 = np.sin(ang).astype(np.float32)
    pairs = (np.arange(128) % 64) // 2
    signs = np.where(np.arange(128) % 2 == 0, -1.0, 1.0).astype(np.float32)
    cosd = np.ascontiguousarray(np.tile(cos1[pairs], (1, B)))
    sind = np.ascontiguousarray(np.tile(sin1[pairs] * signs[:, None], (1, B)))

    k_i = np.arange(128)[:, None]
    j_i = np.arange(256)[None, :]
    maskA = np.where(k_i > j_i, NEG, 0.0).astype(np.float32)
    maskB = np.where(k_i + 128 > j_i, NEG, 0.0).astype(np.float32)
    onesd = np.ones((128, 64), np.float32)

    wq = np.asarray(wq, np.float32)
    wk = np.asarray(wk, np.float32)
    wv = np.asarray(wv, np.float32)
    in_maps = []
    for c in range(NCORES):
        wqTc = np.ascontiguousarray(wq[256 * c:256 * (c + 1), :].T)
        wkvTc = np.ascontiguousarray(
            np.concatenate(
                [wk[64 * c:64 * (c + 1), :].T, wv[64 * c:64 * (c + 1), :].T], axis=1
            )
        )
        in_maps.append(
            {
                "xT": xT, "wqTc": wqTc, "wkvTc": wkvTc, "woT": woT,
                "cosd": cosd, "sind": sind, "maskA": maskA, "maskB": maskB,
                "onesd": onesd,
            }
        )
    return in_maps


def run(x, wq, wk, wv, wo, trace=False):
    """Build, run on 8 cores, assemble full output. Returns (out, results)."""
    from concourse.bass_utils import run_bass_kernel_spmd

    nc = _build()
    in_maps = _host_inputs(x, wq, wk, wv, wo)
    r = run_bass_kernel_spmd(nc, in_maps, list(range(NCORES)), trace=trace)
    out = np.empty((B, S, D), np.float32)
    for c in range(NCORES):
        b, q = c // 4, c % 4
        out[b, 512 * q:512 * (q + 1), :] = r.results[c]["outT"].T
    return out, r


def kernel(x, wq, wk, wv, wo):
    out, _ = run(x, wq, wk, wv, wo, trace=False)
    return out



# revision 3
# speedup vs baseline: 1.0768x; 1.0768x over previous
"""Trainium2 Bass kernel: GQA attention (B=2, S=2048, D=2048, 32 q-heads,
8 kv-heads, head_dim 64, RoPE interleaved, causal) on 8 NeuronCores.

Sharding: tensor-parallel over heads. Core c owns q-heads 4c..4c+3 (= kv head
c) for BOTH batch elements, computes q/k/v projections + RoPE + causal
attention for those heads over all queries, then an 8-core mesh AllToAll
redistributes attention outputs so core c holds all 2048 head-dims for its
(batch, query-quarter) = (c//4, c%4) row block and computes the output
projection locally. The kernel returns out^T row-shards; the host only
transposes/concatenates.

All matmul inputs are bf16 (PSUM accumulation stays fp32), which halves the
DMA-bound phase-1 stream of x and the AllToAll payload. exp runs on ScalarE
with the 1/sqrt(hd) scale folded into the activation's affine pre-scale.
Softmax denominators come from ones-column matmuls accumulated alongside
attn@V; normalization broadcasts the raw sum with a K=1 matmul FIRST, then
takes the reciprocal on the full [64,512] tile (the old order did a [1,512]
reciprocal that serialized 512 elements through one DVE lane, 3.3us each).
"""

import numpy as np

B, S, D = 2, 2048, 2048
NH, NKV, HD = 32, 8, 64
THETA = 10000.0
NCORES = 8
NEG = -1.0e30

_BUILT = None


def _swap_mask():
    m = []
    for i in range(16):
        m += [2 * i + 1, 2 * i]
    return m


def _build():
    """Build + compile the SPMD Bass program (once per process)."""
    global _BUILT
    if _BUILT is not None:
        return _BUILT

    from contextlib import ExitStack

    import concourse.tile as tile
    from concourse import bacc, mybir

    f32 = mybir.dt.float32
    bf = mybir.dt.bfloat16
    AF = mybir.ActivationFunctionType

    nc = bacc.Bacc(
        "TRN2", target_bir_lowering=False, debug=False, num_devices=NCORES
    )

    xT = nc.dram_tensor("xT", [B, 16, 2, 128, 1024], bf, kind="ExternalInput").ap()
    wqTc = nc.dram_tensor("wqTc", [D, 256], bf, kind="ExternalInput").ap()
    wkvTc = nc.dram_tensor("wkvTc", [D, 128], bf, kind="ExternalInput").ap()
    woT = nc.dram_tensor("woT", [D, D], bf, kind="ExternalInput").ap()
    cosd = nc.dram_tensor("cosd", [128, B * S], bf, kind="ExternalInput").ap()
    sind = nc.dram_tensor("sind", [128, B * S], bf, kind="ExternalInput").ap()
    maskA = nc.dram_tensor("maskA", [128, 256], f32, kind="ExternalInput").ap()
    maskB = nc.dram_tensor("maskB", [128, 256], f32, kind="ExternalInput").ap()
    onesd = nc.dram_tensor("onesd", [128, 64], bf, kind="ExternalInput").ap()
    outT = nc.dram_tensor("outT", [D, 512], f32, kind="ExternalOutput").ap()

    SW = _swap_mask()
    SCALE = 1.0 / float(np.sqrt(HD))

    with tile.TileContext(nc) as tc, ExitStack() as top:
        top.enter_context(
            nc.allow_low_precision(reason="bf16 matmul inputs by design")
        )
        wor = top.enter_context(tc.tile_pool(name="wores", bufs=1))
        wo_t = [wor.tile([128, D], bf, tag=f"wo{e}", name=f"wo{e}") for e in range(8)]

        ph12 = top.enter_context(ExitStack())
        res = ph12.enter_context(tc.tile_pool(name="resident", bufs=1))
        # RoPE'd projections, resident across phases
        qt = [res.tile([128, B * S], bf, tag=f"qt{p}", name=f"qt{p}") for p in range(2)]
        kt = res.tile([128, B * S], bf, tag="kt")  # kv head, duplicated rows
        vt = [res.tile([128, HD + 1], bf, tag=f"vt{i}", name=f"vt{i}") for i in range(2 * 16)]
        mA = res.tile([128, 256], f32, tag="mA")
        mB = res.tile([128, 256], f32, tag="mB")
        ones_t = res.tile([128, 64], bf, tag="ones")

        nc.sync.dma_start(out=mA[:], in_=maskA[:])
        nc.sync.dma_start(out=mB[:], in_=maskB[:])
        nc.sync.dma_start(out=ones_t[:], in_=onesd[:])

        dram = top.enter_context(tc.tile_pool(name="dram", bufs=1, space="DRAM"))
        a2a_in = [dram.tile([8, 256, 256], bf, tag=f"a2ain{h}", name=f"a2ain{h}") for h in range(2)]
        a2a_out = [dram.tile([8, 256, 256], bf, tag=f"a2aout{h}", name=f"a2aout{h}") for h in range(2)]

        # ---------------- phase 1: projections + RoPE -------------------
        with ExitStack() as ph1:
            wres = ph1.enter_context(tc.tile_pool(name="wres", bufs=1))
            cos_t = wres.tile([128, B * S], bf, tag="cos")
            sin_t = wres.tile([128, B * S], bf, tag="sin")
            ident = wres.tile([128, 128], bf, tag="ident")
            wq_t = [wres.tile([128, 256], bf, tag=f"wq{d}", name=f"wq{d}") for d in range(16)]
            wkv_t = [wres.tile([128, 128], bf, tag=f"wkv{d}", name=f"wkv{d}") for d in range(16)]
            for d in range(16):
                nc.sync.dma_start(out=wq_t[d][:], in_=wqTc[128 * d:128 * (d + 1), :])
                nc.sync.dma_start(out=wkv_t[d][:], in_=wkvTc[128 * d:128 * (d + 1), :])
            nc.sync.dma_start(out=cos_t[:], in_=cosd[:])
            nc.sync.dma_start(out=sin_t[:], in_=sind[:])
            from concourse.masks import make_identity

            make_identity(nc, ident[:])

            xp = ph1.enter_context(tc.tile_pool(name="xchunk", bufs=4))
            pp = ph1.enter_context(tc.tile_pool(name="projpsum", bufs=1, space="PSUM"))
            tvp = ph1.enter_context(tc.tile_pool(name="vtpsum", bufs=2, space="PSUM"))
            vstage = ph1.enter_context(tc.tile_pool(name="vstage", bufs=2))
            rtmp = ph1.enter_context(tc.tile_pool(name="ropetmp", bufs=1))

            for bh in range(4):
                b, half = bh // 2, bh % 2
                qcs = [1024 * half, 1024 * half + 512]
                cols = [2048 * b + qc for qc in qcs]
                pq = [
                    [pp.tile([128, 512], f32, tag=f"q{s}{i}", name=f"pq{bh}{s}{i}") for i in range(2)]
                    for s in range(2)
                ]
                pkv = [pp.tile([128, 512], f32, tag=f"kv{s}", name=f"pkv{bh}{s}") for s in range(2)]
                for d in range(16):
                    xt_ = xp.tile([128, 1024], bf, tag="x")
                    nc.sync.dma_start(out=xt_[:], in_=xT[b, d, half])
                    st, sp_ = (d == 0), (d == 15)
                    for s in range(2):
                        xs = xt_[:, 512 * s:512 * s + 512]
                        nc.tensor.matmul(pq[s][0][:], wq_t[d][:, 0:128], xs, start=st, stop=sp_)
                        nc.tensor.matmul(pq[s][1][:], wq_t[d][:, 128:256], xs, start=st, stop=sp_)
                        nc.tensor.matmul(pkv[s][:], wkv_t[d][:], xs, start=st, stop=sp_)

                for s in range(2):
                    col = cols[s]
                    # v first: frees the PE transposes to run while DVE does RoPE
                    vs = vstage.tile([128, 512], bf, tag="vs")
                    nc.vector.tensor_copy(vs[64:128, :], pkv[s][64:128, :])
                    for j in range(4):
                        ptv = tvp.tile([128, HD], bf, tag="tv")
                        nc.tensor.transpose(
                            ptv[:], vs[64:128, 128 * j:128 * (j + 1)], ident[64:128, 64:128]
                        )
                        kb = 4 * (2 * half + s) + j
                        nc.vector.tensor_copy(vt[16 * b + kb][:, 0:HD], ptv[:])
                        nc.sync.dma_start(
                            out=vt[16 * b + kb][:, HD:HD + 1], in_=onesd[:, 0:1]
                        )

                    # RoPE on q head-pairs
                    for p in range(2):
                        ps = pq[s][p]
                        t1 = rtmp.tile([128, 512], f32, tag="t1")
                        nc.vector.tensor_mul(t1[:], ps[:], cos_t[:, col:col + 512])
                        sw = rtmp.tile([128, 512], f32, tag="sw")
                        nc.vector.stream_shuffle(sw[:], ps[:], SW)
                        t2 = rtmp.tile([128, 512], f32, tag="t2")
                        nc.vector.tensor_mul(t2[:], sw[:], sin_t[:, col:col + 512])
                        nc.vector.tensor_add(qt[p][:, col:col + 512], t1[:], t2[:])

                    # RoPE on k (kv psum rows 0:64), then duplicate to rows 64:128
                    t1 = rtmp.tile([128, 512], f32, tag="t1")
                    nc.vector.tensor_mul(t1[0:64, :], pkv[s][0:64, :], cos_t[0:64, col:col + 512])
                    sw = rtmp.tile([128, 512], f32, tag="sw")
                    nc.vector.stream_shuffle(sw[0:64, :], pkv[s][0:64, :], SW)
                    t2 = rtmp.tile([128, 512], f32, tag="t2")
                    nc.vector.tensor_mul(t2[0:64, :], sw[0:64, :], sin_t[0:64, col:col + 512])
                    nc.vector.tensor_add(kt[0:64, col:col + 512], t1[0:64, :], t2[0:64, :])
                    nc.sync.dma_start(
                        out=kt[64:128, col:col + 512], in_=kt[0:64, col:col + 512]
                    )


        for e in range(8):
            nc.sync.dma_start(out=wo_t[e][:], in_=woT[128 * e:128 * (e + 1), :])

        # ---------------- phase 2: causal attention ---------------------
        with ExitStack() as ph2:
            spp = ph2.enter_context(tc.tile_pool(name="scorepsum", bufs=3, space="PSUM"))
            avp = ph2.enter_context(tc.tile_pool(name="avpsum", bufs=2, space="PSUM"))
            esp = ph2.enter_context(tc.tile_pool(name="expsbuf", bufs=3))
            nrm = ph2.enter_context(tc.tile_pool(name="normtmp", bufs=2))

            qv = [
                qt[p][:].rearrange("p (b h u i) -> p b h u i", b=2, h=2, u=4)
                for p in range(2)
            ]

            groups = [
                (b, u, p)
                for u in [0, 2, 1, 3]
                for b in range(B)
                for p in range(2)
            ]

            def make_stage(b, u, p, av):
                nkb = 2 * u + 10

                def scores(kb):
                    kcol = 2048 * b + 128 * kb
                    full = kb <= 2 * u + 1
                    sp = spp.tile([128, 1024], f32, tag="sp", name=f"sp{b}{u}{p}{kb}")
                    for hh in range(2):
                        r0 = 64 * hh
                        lhs = kt[r0:r0 + 64, kcol:kcol + 128]
                        if full:
                            rhs = qv[p][r0:r0 + 64, b, :, u, :]
                            nc.tensor.matmul(
                                sp[:, 512 * hh:512 * hh + 512], lhs, rhs,
                                start=True, stop=True,
                            )
                        else:
                            rhs = qv[p][r0:r0 + 64, b, 1, u, :]
                            nc.tensor.matmul(
                                sp[:, 512 * hh + 256:512 * hh + 512], lhs, rhs,
                                start=True, stop=True,
                            )
                    msk = None
                    if kb == 2 * u:
                        msk, mlo = mA, True
                    elif kb == 2 * u + 1:
                        msk, mlo = mB, True
                    elif kb == 2 * u + 8:
                        msk, mlo = mA, False
                    elif kb == 2 * u + 9:
                        msk, mlo = mB, False
                    if msk is not None:
                        off = 0 if mlo else 256
                        for hh in range(2):
                            c0 = 512 * hh + off
                            nc.vector.tensor_add(
                                sp[:, c0:c0 + 256], sp[:, c0:c0 + 256], msk[:]
                            )
                    return sp

                def expav(kb, sp, first, last):
                    full = kb <= 2 * u + 1
                    ex = esp.tile([128, 1024], bf, tag="ex", name=f"ex{b}{u}{p}{kb}")
                    if full:
                        nc.scalar.activation(ex[:], sp[:], AF.Exp, scale=SCALE)
                    else:
                        spv = sp[:].rearrange("p (h i) -> p h i", h=2)[:, :, 256:512]
                        exv = ex[:].rearrange("p (h i) -> p h i", h=2)[:, :, 256:512]
                        nc.scalar.activation(exv, spv, AF.Exp, scale=SCALE)
                    v_ = vt[16 * b + kb]
                    for hh in range(2):
                        e0 = 512 * hh
                        if full:
                            # start=True clears has_written BANK-wide: only the
                            # very first MM in the tile sets it
                            nc.tensor.matmul(
                                av[hh][:, 0:512], v_[:], ex[:, e0:e0 + 512],
                                start=first, stop=last,
                            )
                        else:
                            nc.tensor.matmul(
                                av[hh][:, 256:512], v_[:], ex[:, e0 + 256:e0 + 512],
                                start=first, stop=last,
                            )

                def avcopy():
                    cps = []
                    for hh in range(2):
                        cp = nrm.tile([65, 512], bf, tag="cp", name=f"cp{b}{u}{p}{hh}")
                        nc.vector.tensor_copy(cp[:], av[hh][0:65, :])
                        cps.append(cp)
                    return cps

                def recip(cps):
                    # broadcast the denominator row with a K=1 matmul FIRST,
                    # then reciprocal the full [64,512] tile (a [1,512]
                    # reciprocal serializes through one DVE lane: 3.3us)
                    rrs = []
                    for hh in range(2):
                        bcp = spp.tile([128, 512], f32, tag="sp", name=f"bc{b}{u}{p}{hh}")
                        nc.tensor.matmul(
                            bcp[0:64, :], ones_t[64:65, 0:64], cps[hh][64:65, :],
                            start=True, stop=True,
                        )
                        rr = nrm.tile([64, 512], bf, tag="rr", name=f"rr{b}{u}{p}{hh}")
                        nc.vector.reciprocal(rr[:], bcp[0:64, :])
                        rrs.append(rr)
                    return rrs

                def rest(cps, rrs):
                    for hh in range(2):
                        at_ = nrm.tile([64, 512], bf, tag="at", name=f"at{b}{u}{p}{hh}")
                        nc.vector.tensor_mul(at_[:], cps[hh][0:64, :], rrs[hh][:])
                        for hf in range(2):
                            dst = 4 * b + 2 * hf + u // 2
                            nc.sync.dma_start(
                                out=a2a_in[u % 2][
                                    dst, 128 * p + 64 * hh:128 * p + 64 * hh + 64, :
                                ],
                                in_=at_[:, 256 * hf:256 * hf + 256],
                            )

                return scores, expav, avcopy, recip, rest, nkb

            def emit_a2a(h):
                nc.gpsimd.collective_compute(
                    "AllToAll",
                    mybir.AluOpType.bypass,
                    replica_groups=[list(range(8))],
                    ins=[a2a_in[h][:].opt()],
                    outs=[a2a_out[h][:].opt()],
                )

            pending_recip = None
            pending_rest = None
            for gi, (b, u, p) in enumerate(groups):
                av = [
                    avp.tile([HD + 1, 512], f32, tag="av", name=f"av{b}{u}{p}{hh}")
                    for hh in range(2)
                ]
                scores, expav, avcopy, recip, rest, nkb = make_stage(b, u, p, av)
                diag = [2 * u, 2 * u + 1, 2 * u + 8, 2 * u + 9]
                order = [k for k in range(nkb) if k not in diag] + diag
                pipe = []
                for i, kb in enumerate(order):
                    sp = scores(kb)
                    pipe.append((kb, sp))
                    if i == 2 and pending_recip is not None:
                        rrs = pending_recip()
                        pending_recip = None
                        pending_rest = (lambda rrs=rrs: _rest_holder[0](rrs))
                    if i == 8 and pending_rest is not None:
                        pending_rest()
                        pending_rest = None
                        if gi == 8:
                            emit_a2a(0)  # first-half attn outputs all written
                    if len(pipe) > 2:
                        pk, psp = pipe.pop(0)
                        expav(pk, psp, first=(pk == order[0]), last=(pk == order[-1]))
                for pk, psp in pipe:
                    expav(pk, psp, first=(pk == order[0]), last=(pk == order[-1]))
                cps = avcopy()
                _rest_holder = [rest]
                pending_recip = (lambda recip=recip, rest=rest, cps=cps: recip(cps))
                _rest_holder[0] = (lambda rest=rest, cps=cps: (lambda rrs: rest(cps, rrs)))()
            rrs = pending_recip()
            _rest_holder[0](rrs)
            emit_a2a(1)


        ph12.close()

        # ---------------- phase 3: output projection --------------------
        with ExitStack() as ph3:
            wrh = ph3.enter_context(tc.tile_pool(name="worh", bufs=1))
            wo_t2 = [wrh.tile([128, D], bf, tag=f"wo2{e}", name=f"wo2{e}") for e in range(8)]
            rh_t = [
                [wrh.tile([128, 256], bf, tag=f"rh{h}{e}", name=f"rh{h}{e}") for e in range(16)]
                for h in range(2)
            ]
            for e in range(16):
                nc.sync.dma_start(
                    out=rh_t[0][e][:],
                    in_=a2a_out[0][e // 2, 128 * (e % 2):128 * (e % 2) + 128, :],
                )
            for e in range(8):
                nc.sync.dma_start(out=wo_t2[e][:], in_=woT[128 * (e + 8):128 * (e + 9), :])
            for e in range(16):
                nc.sync.dma_start(
                    out=rh_t[1][e][:],
                    in_=a2a_out[1][e // 2, 128 * (e % 2):128 * (e % 2) + 128, :],
                )
            wo_t = wo_t + wo_t2
            wop = ph3.enter_context(tc.tile_pool(name="wopsum", bufs=2, space="PSUM"))
            wos = ph3.enter_context(tc.tile_pool(name="wosbuf", bufs=2))
            warm0 = wop.tile([128, 256], f32, tag="warm", name="warmbank0")
            for w in range(56):
                nc.tensor.matmul(
                    warm0[:], wo_t[0][:, 0:128], rh_t[0][0][:],
                    start=True, stop=True, skip_group_check=True,
                )
            for h in range(2):
                if h == 1:
                    # keep the PE clock warm (HAM) across the AllToAll-2 wait;
                    # junk matmuls into a scratch bank, no consumers
                    warm = wop.tile([128, 256], f32, tag="warm", name="warmbank")
                    for w in range(120):
                        nc.tensor.matmul(
                            warm[:], wo_t[0][:, 0:128], rh_t[0][0][:],
                            start=True, stop=True, skip_group_check=True,
                        )
                for m in range(16):
                    po = wop.tile([128, 256], f32, tag="wo")
                    for e in range(16):
                        nc.tensor.matmul(
                            po[:], wo_t[e][:, 128 * m:128 * (m + 1)], rh_t[h][e][:],
                            start=(e == 0), stop=(e == 15),
                        )
                    os_ = wos.tile([128, 256], f32, tag="os")
                    nc.vector.tensor_copy(os_[:], po[:])
                    nc.sync.dma_start(
                        out=outT[128 * m:128 * (m + 1), 256 * h:256 * h + 256], in_=os_[:]
                    )

    nc.compile()
    _BUILT = nc
    return nc


def _host_inputs(x, wq, wk, wv, wo):
    """Per-core input maps (host-side layout prep only, no math on x)."""
    import ml_dtypes

    bf16 = ml_dtypes.bfloat16
    x = np.ascontiguousarray(x, dtype=np.float32)
    xT3 = x.transpose(0, 2, 1)
    xT = np.ascontiguousarray(
        xT3.reshape(B, 16, 128, 2, 1024).transpose(0, 1, 3, 2, 4).astype(bf16)
    )
    woT = np.ascontiguousarray(np.asarray(wo, np.float32).T.astype(bf16))

    inv = THETA ** (-np.arange(32, dtype=np.float64) / 32.0)
    ang = np.outer(inv, np.arange(S, dtype=np.float64))  # [32, S]
    cos1 = np.cos(ang).astype(np.float32)
    sin1 = np.sin(ang).astype(np.float32)
    pairs = (np.arange(128) % 64) // 2
    signs = np.where(np.arange(128) % 2 == 0, -1.0, 1.0).astype(np.float32)
    cosd = np.ascontiguousarray(np.tile(cos1[pairs], (1, B)).astype(bf16))
    sind = np.ascontiguousarray(
        np.tile(sin1[pairs] * signs[:, None], (1, B)).astype(bf16)
    )

    k_i = np.arange(128)[:, None]
    j_i = np.arange(256)[None, :]
    maskA = np.where(k_i > j_i, NEG, 0.0).astype(np.float32)
    maskB = np.where(k_i + 128 > j_i, NEG, 0.0).astype(np.float32)
    onesd = np.ones((128, 64), bf16)

    wq = np.asarray(wq, np.float32)
    wk = np.asarray(wk, np.float32)
    wv = np.asarray(wv, np.float32)
    in_maps = []
    for c in range(NCORES):
        wqTc = np.ascontiguousarray(wq[256 * c:256 * (c + 1), :].T.astype(bf16))
        wkvTc = np.ascontiguousarray(
            np.concatenate(
                [wk[64 * c:64 * (c + 1), :].T, wv[64 * c:64 * (c + 1), :].T], axis=1
            ).astype(bf16)
        )
        in_maps.append(
            {
                "xT": xT, "wqTc": wqTc, "wkvTc": wkvTc, "woT": woT,
                "cosd": cosd, "sind": sind, "maskA": maskA, "maskB": maskB,
                "onesd": onesd,
            }
        )
    return in_maps


def run(x, wq, wk, wv, wo, trace=False):
    """Build, run on 8 cores, assemble full output. Returns (out, results)."""
    from concourse.bass_utils import run_bass_kernel_spmd

    nc = _build()
    in_maps = _host_inputs(x, wq, wk, wv, wo)
    r = run_bass_kernel_spmd(nc, in_maps, list(range(NCORES)), trace=trace)
    out = np.empty((B, S, D), np.float32)
    for c in range(NCORES):
        b, q = c // 4, c % 4
        out[b, 512 * q:512 * (q + 1), :] = r.results[c]["outT"].T
    return out, r


def kernel(x, wq, wk, wv, wo):
    out, _ = run(x, wq, wk, wv, wo, trace=False)
    return out
